# revision 17
# baseline (speedup 1.0000x reference)
"""Causal multi-head attention kernel for TRN2 (8 NeuronCores, SPMD).

Problem: x[2,2048,1024], per-head W_qkv[16,1024,192], W_out[16,64,1024].
  qkv = einsum('bsd,ndh->bnsh', x, W_qkv); causal softmax attention per head;
  out.reshape(B,-1,S); einsum('bds,nhd->bsd', out, W_out).

The final einsum does not contract d, so it reduces to a per-column scale by
W_sum[d] = sum_{n,h} W_out[n,h,d]; that part runs on the host.  The device
computes the attention for 4 heads x 1 batch per core (2 head-pairs packed
into 128 partitions).

v2 (this file): same math as v1, rescheduled for the real bottleneck
structure measured on HW:
  - attention steady-state is ScalarE(exp)-bound; projection phase left
    ScalarE idle ~36us and block boundaries idled the PE ~19us.
  - Rework: one flat software-pipelined stream.  Per k-step: scores pair
    (row-tiled tile_position matmuls), the AV pair trailing >=2 steps, plus
    ~2.4 projection/transpose "quanta" pulled from a generator.  Blocks
    pipeline into each other with no boundary stalls.
  - Startup: DMA in exact consumption order, small chunks; first projection
    matmul can start after ~0.4MB instead of ~5MB.
  - PSUM: s2 double-buffered (2x2 banks) + oa + ob + 2 work banks = 8.
  - Output: per-block stage copy then 4-way-chunked DMA so the tail is short.
"""

import numpy as np

import concourse.bass as bass
import concourse.mybir as mybir
from concourse.tile import TileContext
from concourse.bass_utils import run_bass_kernel_spmd

F32 = mybir.dt.float32
MMD = mybir.dt.float16  # matmul operand dtype
NPD = np.float16

B, S, D, NH, HD = 2, 2048, 1024, 16, 64  # batch, seq, model, heads, head_dim
NCORES = 8
HPC = 4  # heads per core
NPAIR = 2  # head pairs per core
DT = D // 128  # 8 D-tiles
NKT = S // 128  # 16 k tiles
SCALE = 1.0 / np.sqrt(HD)


def _split_excess_waits(nc, limit=1):
    """This walrus build rejects >1 sync-wait per instruction; hoist extra
    waits onto preceding same-engine no-ops (identical blocking semantics)."""
    cnt = 0
    for fn in nc.m.functions:
        for blk in fn.blocks:
            out = []
            for inst in blk.instructions:
                si = inst.sync_info
                if si is not None and si.on_wait and len(si.on_wait) > limit:
                    waits = list(si.on_wait)
                    excess, keep = waits[:-limit], waits[-limit:]
                    for i in range(0, len(excess), limit):
                        nop = mybir.InstNoOp(
                            name=f"wsplit_{cnt}", ins=[], outs=[], engine=inst.engine
                        )
                        cnt += 1
                        nop.sync_info = mybir.SyncInfo(
                            on_wait=excess[i : i + limit], on_update=[]
                        )
                        out.append(nop)
                    inst.sync_info = mybir.SyncInfo(
                        on_wait=keep, on_update=list(si.on_update or [])
                    )
                out.append(inst)
            blk.instructions = out
    return cnt


def build_nc():
    nc = bass.Bass()
    xT = nc.declare_dram_parameter("xT", [D, S], MMD, isOutput=False)
    w = nc.declare_dram_parameter("w", [NPAIR, 3, DT, 128, 128], MMD, isOutput=False)
    mask = nc.declare_dram_parameter("mask", [4, 128, 1024], MMD, isOutput=False)
    ident = nc.declare_dram_parameter("ident", [128, 128], MMD, isOutput=False)
    out = nc.declare_dram_parameter("out", [65, HPC * S], F32, isOutput=True)

    with TileContext(nc) as tc:
        with (
            tc.tile_pool(name="persist", bufs=1) as pp,
            tc.tile_pool(name="ps_s2", bufs=2, space="PSUM") as ps_s2,
            tc.tile_pool(name="ps_oa", bufs=1, space="PSUM") as ps_oa,
            tc.tile_pool(name="ps_ob", bufs=1, space="PSUM") as ps_ob,
            tc.tile_pool(name="ps_wk", bufs=2, space="PSUM") as ps_wk,
            tc.tile_pool(name="ptp", bufs=24) as ptp,
            tc.tile_pool(name="stg", bufs=2) as stg,
        ):
            # ---- persistent SBUF tensors
            qt2 = [pp.tile([128, S], MMD, tag=f"qt{p}", name=f"qtt{p}") for p in range(NPAIR)]
            kt2 = [pp.tile([128, S], MMD, tag=f"kt{p}", name=f"ktt{p}") for p in range(NPAIR)]
            vt = [pp.tile([128, S], MMD, tag=f"vt{p}", name=f"vtt{p}") for p in range(NPAIR)]
            v2e = [
                pp.tile([128, NKT, 130], MMD, tag=f"v2e{p}", name=f"v2e{p}")
                for p in range(NPAIR)
            ]
            mask_sb = pp.tile([128, 4, 1024], MMD, tag="mask", name="mask_sb")
            ident_sb = pp.tile([128, 128], MMD, tag="ident", name="ident_sb")
            xt_sb = pp.tile([128, DT, S], MMD, tag="xt", name="xt_sb")
            w_sb = pp.tile([128, NPAIR * 3 * DT, 128], MMD, tag="w", name="w_sb")

            w_v = w.rearrange("a t d k m -> k (a t d) m")
            xt_v = xT.rearrange("(dt p) s -> p dt s", p=128)

            # ---- DMA in consumption order, small chunks for queue
            # parallelism.  Phase A consumes: ident (warmup), w(p0,q),
            # xt cols 512:1024, then w(p0,k) + xt cols 0:512.
            # First the minimal working set of the very first matmuls (xt d0
            # pair + w q d0:4), in smallest-first order so the PE can start
            # dribbling accumulation matmuls at ~9us; the dribble also trips
            # the HAM clock gate before the dense phase begins.
            nc.sync.dma_start(out=xt_sb[:, 0:2, 0:512], in_=xt_v[:, 0:2, 0:512])
            nc.sync.dma_start(out=w_sb[:, 0:4, :], in_=w_v[:, 0:4, :])
            nc.sync.dma_start(out=xt_sb[:, 2:4, 0:512], in_=xt_v[:, 2:4, 0:512])
            nc.sync.dma_start(out=w_sb[:, 4:8, :], in_=w_v[:, 4:8, :])
            nc.sync.dma_start(out=ident_sb[:], in_=ident[:])
            nc.sync.dma_start(out=xt_sb[:, 4:6, 0:512], in_=xt_v[:, 4:6, 0:512])
            nc.sync.dma_start(out=xt_sb[:, 6:8, 0:512], in_=xt_v[:, 6:8, 0:512])
            for h in range(2):
                nc.sync.dma_start(
                    out=w_sb[:, DT + 4 * h : DT + 4 * h + 4, :],
                    in_=w_v[:, DT + 4 * h : DT + 4 * h + 4, :],
                )
            for d2 in range(4):
                nc.sync.dma_start(
                    out=xt_sb[:, 2 * d2 : 2 * d2 + 2, 512:1024],
                    in_=xt_v[:, 2 * d2 : 2 * d2 + 2, 512:1024],
                )
            mask_v = mask.rearrange("r k q -> k r q")
            nc.sync.dma_start(out=mask_sb[:, 0:2, :], in_=mask_v[:, 0:2, :])
            nc.sync.dma_start(out=mask_sb[:, 2:4, :], in_=mask_v[:, 2:4, :])
            nc.sync.dma_start(out=w_sb[:, 2 * DT : 3 * DT, :], in_=w_v[:, 2 * DT : 3 * DT, :])
            for c4 in (2, 3):
                for d2 in range(4):
                    nc.sync.dma_start(
                        out=xt_sb[:, 2 * d2 : 2 * d2 + 2, c4 * 512 : (c4 + 1) * 512],
                        in_=xt_v[:, 2 * d2 : 2 * d2 + 2, c4 * 512 : (c4 + 1) * 512],
                    )
            nc.sync.dma_start(out=w_sb[:, 3 * DT : 6 * DT, :], in_=w_v[:, 3 * DT : 6 * DT, :])
            for p in range(NPAIR):
                nc.gpsimd.memset(v2e[p][:, :, 64], 1.0)
                nc.gpsimd.memset(v2e[p][:, :, 129], 1.0)

            DST = {0: qt2, 1: kt2, 2: vt}

            # ---- projection chunk machinery.  One chunk = 512 q-columns of
            # one (pair, q/k/v).  q/k chunks: 8 matmuls + cast.  v chunks:
            # 8 matmuls + cast + 4 transposes + v2e copies.
            ready = {"qk": set(), "v2e": set()}  # (p, t, qc) / (p, ktile)

            def emit_chunk_mms(p, t, qc, acc):
                for d in range(DT):
                    yield ("pe", lambda acc=acc, p=p, t=t, qc=qc, d=d: nc.tensor.matmul(
                        acc[:],
                        w_sb[:, (p * 3 + t) * DT + d, :],
                        xt_sb[:, d, qc * 512 : (qc + 1) * 512],
                        start=(d == 0),
                        stop=(d == DT - 1),
                    ))

            def chunk_quanta(p, t, qc):
                acc = ps_wk.tile([128, 512], F32, tag="wk", name=f"acc{p}{t}{qc}")
                yield from emit_chunk_mms(p, t, qc, acc)

                def cast(acc=acc, p=p, t=t, qc=qc):
                    nc.vector.tensor_copy(
                        DST[t][p][:, qc * 512 : (qc + 1) * 512], acc[:]
                    )
                    ready["qk"].add((p, t, qc))
                yield ("dve", cast)

                if t == 2:
                    tp = ps_wk.tile([128, 4, 128], MMD, tag="wk", name=f"tp{p}{qc}")
                    for i in range(4):
                        def tr(tp=tp, p=p, qc=qc, i=i):
                            k = 4 * qc + i
                            nc.tensor.transpose(
                                tp[:, i, :],
                                vt[p][:, k * 128 : (k + 1) * 128],
                                ident_sb[:],
                            )
                        yield ("pe", tr)

                    def cpv(tp=tp, p=p, qc=qc):
                        nc.vector.tensor_copy(
                            v2e[p][:, 4 * qc : 4 * qc + 4, 0:64], tp[:, :, 0:64]
                        )
                        nc.vector.tensor_copy(
                            v2e[p][:, 4 * qc : 4 * qc + 4, 65:129], tp[:, :, 64:128]
                        )
                        for k in range(4 * qc, 4 * qc + 4):
                            ready["v2e"].add((p, k))
                    yield ("dve", cpv)

            def pair_quanta(p, t, qcp):
                """Both 512-col chunks of (p, t) for q-column-pair qcp, d-major
                so each weight is loaded once and streams two matmuls."""
                qc0, qc1 = 2 * qcp, 2 * qcp + 1
                acc0 = ps_wk.tile([128, 512], F32, tag="wk", name=f"pacc0{p}{t}{qcp}")
                acc1 = ps_wk.tile([128, 512], F32, tag="wk", name=f"pacc1{p}{t}{qcp}")
                for d in range(DT):
                    def mm2(p=p, t=t, d=d, acc0=acc0, acc1=acc1, qc0=qc0, qc1=qc1):
                        wsl = w_sb[:, (p * 3 + t) * DT + d, :]
                        for acc, qc in ((acc0, qc0), (acc1, qc1)):
                            nc.tensor.matmul(
                                acc[:],
                                wsl,
                                xt_sb[:, d, qc * 512 : (qc + 1) * 512],
                                start=(d == 0),
                                stop=(d == DT - 1),
                            )
                    yield ("pe2", mm2)
                for acc, qc in ((acc0, qc0), (acc1, qc1)):
                    def cast(acc=acc, p=p, t=t, qc=qc):
                        nc.vector.tensor_copy(
                            DST[t][p][:, qc * 512 : (qc + 1) * 512], acc[:]
                        )
                        ready["qk"].add((p, t, qc))
                    yield ("dve", cast)
                if t == 2:
                    tp = ps_wk.tile([128, 8, 128], MMD, tag="wk", name=f"ptp{p}{qcp}")
                    for i in range(8):
                        def tr(tp=tp, p=p, qcp=qcp, i=i):
                            k = 8 * qcp + i
                            nc.tensor.transpose(
                                tp[:, i, :],
                                vt[p][:, k * 128 : (k + 1) * 128],
                                ident_sb[:],
                            )
                        yield ("pe", tr)

                    def cpv(tp=tp, p=p, qcp=qcp):
                        nc.vector.tensor_copy(
                            v2e[p][:, 8 * qcp : 8 * qcp + 8, 0:64], tp[:, :, 0:64]
                        )
                        nc.vector.tensor_copy(
                            v2e[p][:, 8 * qcp : 8 * qcp + 8, 65:129], tp[:, :, 64:128]
                        )
                        for k in range(8 * qcp, 8 * qcp + 8):
                            ready["v2e"].add((p, k))
                    yield ("dve", cpv)

            # ---- phase A: q and k cols 0:512 of pair 0 — the working set of
            # block (0, qb=0), which shares a single 1MB xT chunk — up front.
            for kind, fn in chunk_quanta(0, 0, 0):
                fn()
            for kind, fn in chunk_quanta(0, 1, 0):
                fn()

            def proj_quanta():
                yield from chunk_quanta(0, 0, 1)
                yield from chunk_quanta(0, 1, 1)
                pairs = [
                    (0, 0, 1), (0, 1, 1), (0, 2, 0), (0, 2, 1),
                    (1, 0, 0), (1, 0, 1), (1, 1, 0), (1, 1, 1), (1, 2, 0), (1, 2, 1),
                ]
                for p, t, qcp in pairs:
                    yield from pair_quanta(p, t, qcp)

            gen = proj_quanta()
            gen_done = False
            pe_pulled = 0

            def pull(n_pe):
                """Emit quanta until n_pe more 216ns-units of PE work are out."""
                nonlocal gen_done, pe_pulled
                got = 0
                while got < n_pe and not gen_done:
                    try:
                        kind, fn = next(gen)
                    except StopIteration:
                        gen_done = True
                        break
                    fn()
                    if kind == "pe":
                        got += 1
                        pe_pulled += 1
                    elif kind == "pe2":
                        got += 2
                        pe_pulled += 2

            def need(p, t, qc):
                while (p, t, qc) not in ready["qk"] and not gen_done:
                    pull(1)

            # ---- flat attention stream
            block_order = [(0, 0), (0, 1), (0, 3), (0, 2), (1, 1), (1, 0), (1, 3), (1, 2)]
            steps = []
            for p, qb in block_order:
                nk = 4 * (qb + 1)
                for k in range(nk):
                    steps.append((p, qb, k, nk))
            nsteps = len(steps)

            blk_state = {}  # (p, qb) -> dict(oa, ob, pts)
            av_queue = []  # (p, qb, k, nk, step_emitted)

            def scores(p, qb, k, nk):
                st = blk_state.setdefault((p, qb), {"pts": {}})
                q0 = max(0, 128 * (k - 4 * qb))
                s2 = ps_s2.tile([128, 1024], F32, tag="s2", name="s2")
                qsl = slice(qb * 512 + q0, (qb + 1) * 512)
                for e in range(2):
                    rows = slice(64 * e, 64 * e + 64)
                    nc.tensor.matmul(
                        s2[:, e * 512 + q0 : (e + 1) * 512],
                        kt2[p][rows, k * 128 : (k + 1) * 128],
                        qt2[p][rows, qsl],
                        start=True,
                        stop=True,
                        tile_position=(64 * e, 0),
                    )
                pt2 = ptp.tile([128, 1024], MMD, tag="pt", name="pt2")
                nc.scalar.activation(
                    pt2[:, q0:1024],
                    s2[:, q0:1024],
                    mybir.ActivationFunctionType.Exp,
                    scale=float(SCALE),
                )
                rel = k - 4 * qb
                if rel >= 0:  # diagonal-crossing: 0/1 mask
                    nc.vector.tensor_mul(
                        pt2[:, q0:1024],
                        pt2[:, q0:1024],
                        mask_sb[:, rel, q0:1024],
                    )
                st["pts"][k] = (pt2, q0)

            def av(p, qb, k, nk):
                st = blk_state[(p, qb)]
                if k == 0:
                    st["oa"] = ps_oa.tile([65, 512], F32, tag="oa", name="oa")
                    st["ob"] = ps_ob.tile([65, 512], F32, tag="ob", name="ob")
                pt2, q0 = st["pts"].pop(k)
                nc.tensor.matmul(
                    st["oa"][:, q0:512],
                    v2e[p][:, k, 0:65],
                    pt2[:, q0:512],
                    start=(k == 0),
                    stop=(k == nk - 1),
                )
                nc.tensor.matmul(
                    st["ob"][:, q0:512],
                    v2e[p][:, k, 65:130],
                    pt2[:, 512 + q0 : 1024],
                    start=(k == 0),
                    stop=(k == nk - 1),
                )
                if k == nk - 1:
                    finish_block(p, qb)

            out_v = out.rearrange("h (nl q) -> h nl q", nl=HPC)

            def finish_block(p, qb):
                st = blk_state.pop((p, qb))
                stage = stg.tile([65, 1024], F32, tag="stage", name="stage")
                nc.vector.tensor_copy(stage[:, 0:512], st["ob"][:])
                nc.vector.tensor_copy(stage[:, 512:1024], st["oa"][:])
                # chunked output DMA (spread queues, short tail)
                for e in range(2):
                    src = stage[:, (1 - e) * 512 : (2 - e) * 512]
                    for j in range(2):
                        nc.sync.dma_start(
                            out=out_v[
                                :, 2 * p + e, qb * 512 + j * 256 : qb * 512 + (j + 1) * 256
                            ],
                            in_=src[:, j * 256 : (j + 1) * 256],
                        )

            AV_LAG = 2
            RATE = 4.0  # pe quanta per step

            for s, (p, qb, k, nk) in enumerate(steps):
                # make sure this step's q/k chunks exist before its scores
                need(p, 0, qb)
                need(p, 1, k // 4)
                scores(p, qb, k, nk)
                av_queue.append((p, qb, k, nk, s))
                # drain ready avs (lagged >= AV_LAG steps, v2e present)
                while av_queue:
                    ap_, aqb, ak, ank, as_ = av_queue[0]
                    if s - as_ < AV_LAG or (ap_, ak) not in ready["v2e"]:
                        break
                    av_queue.pop(0)
                    av(ap_, aqb, ak, ank)
                # uniform projection quantum rate
                want = int(RATE * (s + 1)) - pe_pulled
                if want > 0:
                    pull(want)

            while not gen_done:
                pull(4)
            while av_queue:
                ap_, aqb, ak, ank, as_ = av_queue.pop(0)
                av(ap_, aqb, ak, ank)

    _split_excess_waits(nc)
    return nc


_NC_CACHE = None


def _get_nc():
    global _NC_CACHE
    if _NC_CACHE is None:
        _NC_CACHE = build_nc()
    return _NC_CACHE


def _host_inputs(x, W_qkv):
    """Per-core input maps."""
    xT = [np.ascontiguousarray(x[b].T).astype(NPD) for b in range(B)]  # [D, S]
    # w[pair, t, dtile, 128, 128]: cols 0:64 head a, 64:128 head b
    Wr = np.ascontiguousarray(W_qkv.reshape(NH, DT, 128, 3, HD))
    ki = np.arange(128)[:, None]
    qj = np.arange(512)[None, :]
    m1 = np.zeros((4, 128, 512), dtype=np.float32)
    for r in range(4):
        m1[r] = (ki <= qj - 128 * r).astype(np.float32)
    mask = np.concatenate([m1, m1], axis=2).astype(NPD)  # [4, 128, 1024]
    ident = np.eye(128, dtype=np.float32).astype(NPD)
    in_maps = []
    for c in range(NCORES):
        b = c // 4
        h0 = 4 * (c % 4)
        wm = np.empty((NPAIR, 3, DT, 128, 128), dtype=np.float32)
        for p in range(NPAIR):
            ha, hb = h0 + 2 * p, h0 + 2 * p + 1
            for t in range(3):
                wm[p, t, :, :, 0:64] = Wr[ha, :, :, t, :]
                wm[p, t, :, :, 64:128] = Wr[hb, :, :, t, :]
        in_maps.append(
            {"xT": xT[b], "w": wm.astype(NPD), "mask": mask, "ident": ident}
        )
    return in_maps


def _host_epilogue(results, W_out):
    W_sum = W_out.sum(axis=(0, 1)).astype(np.float32)  # [D]
    O = np.empty((B, NH, S, HD), dtype=np.float32)
    for c in range(NCORES):
        o = results[c]["out"]  # [65, 4*2048]
        b = c // 4
        h0 = 4 * (c % 4)
        body = o[0:64].reshape(64, HPC, S)  # [h, nl, s]
        den = o[64].reshape(HPC, S)  # [nl, s]
        O[b, h0 : h0 + HPC] = body.transpose(1, 2, 0) / den[:, :, None]
    out2 = O.reshape(B, D, S)  # raw row-major reshape, as in the reference
    return np.ascontiguousarray(
        out2.transpose(0, 2, 1) * W_sum[None, None, :]
    ).astype(np.float32)


def _run(x, W_qkv, W_out, trace=False):
    nc = _get_nc()
    in_maps = _host_inputs(x, W_qkv)
    res = run_bass_kernel_spmd(
        nc,
        in_maps,
        list(range(NCORES)),
        trace=trace,
        trace_cores=list(range(NCORES)) if trace else None,
    )
    return _host_epilogue(res.results, W_out), res


def kernel(x, W_qkv, W_out):
    x = np.asarray(x, dtype=np.float32)
    W_qkv = np.asarray(W_qkv, dtype=np.float32)
    W_out = np.asarray(W_out, dtype=np.float32)
    out, _ = _run(x, W_qkv, W_out, trace=False)
    return out


def kernel_traced(x, W_qkv, W_out):
    out, res = _run(
        np.asarray(x, np.float32),
        np.asarray(W_qkv, np.float32),
        np.asarray(W_out, np.float32),
        trace=True,
    )
    return out, res
